# revision 10
# baseline (speedup 1.0000x reference)
"""Trainium2 Bass kernel for nn_MetaLayer: per-sample fast-weight update.

Data parallel over B=8 (one sample per NeuronCore). Per core:
  fwd in_ff (swiglu) -> fwd predictor_ff -> cosine-loss grad ->
  backprop to in_ff params -> W' = u_ff - |lr|*grad -> re-forward -> layernorm.

Layout: feature-major activations (X^T tiles [feat128, tok]) for fwd/bwd;
token-contract matmuls for dW via DMA-xbar transposed reads of bf16 scratch;
token-major re-forward with PE-transposed u'. Matmuls bf16, fp32 PSUM.
"""
import os
import sys as _sys
import numpy as np
import ml_dtypes

import concourse.bass as bass
import concourse.mybir as mybir
from concourse.tile import TileContext
from concourse.bass_utils import run_bass_kernel_spmd

_sys.path.insert(0, os.path.dirname(os.path.abspath(__file__)))
import tile_fix

F32 = mybir.dt.float32
BF16 = mybir.dt.bfloat16
AF = mybir.ActivationFunctionType
ALU = mybir.AluOpType
AX = mybir.AxisListType

B, N, D, G, GD = 8, 2048, 1024, 16, 64
D2, D4 = 2 * D, 4 * D
NT = 512            # token chunk (free dim) for phase A
NCHUNK = N // NT    # 4
TTILES = N // 128   # 16 token tiles


def _consts():
    ident = np.eye(128, dtype=ml_dtypes.bfloat16)
    ones1b = np.ones((1, 128), dtype=ml_dtypes.bfloat16)
    ones1f = np.ones((1, 128), dtype=np.float32)
    # IND[j]: [128, 16], feat-tile j of D covers groups 2j (p<64), 2j+1 (p>=64)
    ind = np.zeros((8, 128, 16), dtype=ml_dtypes.bfloat16)
    for j in range(8):
        ind[j, :64, 2 * j] = 1
        ind[j, 64:, 2 * j + 1] = 1
    indt = np.transpose(ind, (0, 2, 1)).copy()
    return ident, ones1b, ones1f, ind, indt


def build_nc():
    nc = bass.Bass()
    def inp(name, shape, dt=F32):
        return nc.declare_dram_parameter(name, list(shape), dt, isOutput=False)

    xs_h = inp("xs", (N, D)); tg_h = inp("tg", (N, D))
    W1_h = inp("W1", (D, D4)); b1_h = inp("b1", (D4,))
    W2_h = inp("W2", (D2, D)); b2_h = inp("b2", (D,))
    pW1_h = inp("pW1", (D, D2)); pb1_h = inp("pb1", (D2,))
    pW2_h = inp("pW2", (D, D)); pb2_h = inp("pb2", (D,))
    uW1_h = inp("uW1", (D, D4)); ub1_h = inp("ub1", (D4,))
    uW2_h = inp("uW2", (D2, D)); ub2_h = inp("ub2", (D,))
    lr_h = inp("lr", (1, 1))
    lng_h = inp("lng", (D,)); lnb_h = inp("lnb", (D,))
    ident_h = inp("ident", (128, 128), BF16)
    ones1b_h = inp("ones1b", (1, 128), BF16)
    ones1f_h = inp("ones1f", (1, 128))
    ind_h = inp("ind", (8 * 128, 16), BF16)
    indt_h = inp("indt", (8 * 16, 128), BF16)

    Y = nc.declare_dram_parameter("y", [N, D], F32, isOutput=True)
    DEBUG = os.environ.get("KERNEL_DEBUG", "0") == "1"
    dbg = {}
    def dbgout(name, shape):
        dbg[name] = nc.declare_dram_parameter("dbg_" + name, list(shape), BF16, isOutput=True)

    # bf16 DRAM scratch
    xs_t_s = nc.dram_tensor("xs_t_s", [N, D], BF16)
    tg_t_s = nc.dram_tensor("tg_t_s", [N, D], BF16)
    s_f_s = nc.dram_tensor("s_f_s", [D2, N], BF16)
    gt_f_s = nc.dram_tensor("gt_f_s", [D2, N], BF16)
    sgp_f_s = nc.dram_tensor("sgp_f_s", [D2, N], BF16)
    u_f_s = nc.dram_tensor("u_f_s", [D2, N], BF16)
    h_f_s = nc.dram_tensor("h_f_s", [D, N], BF16)
    dh_f_s = nc.dram_tensor("dh_f_s", [D, N], BF16)
    dH1_f_s = nc.dram_tensor("dH1_f_s", [D4, N], BF16)
    s2_f_s = nc.dram_tensor("s2_f_s", [D, N], BF16)
    g2_f_s = nc.dram_tensor("g2_f_s", [D, N], BF16)
    sgp2_f_s = nc.dram_tensor("sgp2_f_s", [D, N], BF16)
    dpr_f_s = nc.dram_tensor("dpr_f_s", [D, N], BF16)
    w2_s = nc.dram_tensor("w2_s", [D2, D], BF16)
    pw1_s = nc.dram_tensor("pw1_s", [D, D2], BF16)
    pw2_s = nc.dram_tensor("pw2_s", [D, D], BF16)
    w1p_s = nc.dram_tensor("w1p_s", [D, D4], BF16)
    w2p_s = nc.dram_tensor("w2p_s", [D2, D], BF16)
    b1p_s = nc.dram_tensor("b1p_s", [D4], BF16)
    b2p_s = nc.dram_tensor("b2p_s", [D], BF16)

    dma = nc.sync

    with TileContext(nc) as tc:
        with tc.tile_pool(name="persist", bufs=1) as pp:
            # ------- small persistent constants -------
            ident = pp.tile([128, 128], BF16)
            dma.dma_start(out=ident, in_=ident_h[:])
            ones1b = pp.tile([1, 128], BF16)
            dma.dma_start(out=ones1b, in_=ones1b_h[:])
            ones1f = pp.tile([1, 128], F32)
            dma.dma_start(out=ones1f, in_=ones1f_h[:])
            ind = pp.tile([128, 8, 16], BF16)
            dma.dma_start(out=ind, in_=ind_h.rearrange("(j p) g -> p j g", p=128))
            indt = pp.tile([16, 8, 128], BF16)
            dma.dma_start(out=indt, in_=indt_h.rearrange("(j g) p -> g j p", g=16))

            b1c = pp.tile([128, 32], F32)
            dma.dma_start(out=b1c, in_=b1_h.rearrange("(t p) -> p t", p=128))
            b2c = pp.tile([128, 8], F32)
            dma.dma_start(out=b2c, in_=b2_h.rearrange("(t p) -> p t", p=128))
            pb1c = pp.tile([128, 16], F32)
            dma.dma_start(out=pb1c, in_=pb1_h.rearrange("(t p) -> p t", p=128))
            pb2c = pp.tile([128, 8], F32)
            dma.dma_start(out=pb2c, in_=pb2_h.rearrange("(t p) -> p t", p=128))
            ub1c = pp.tile([128, 32], F32)
            dma.dma_start(out=ub1c, in_=ub1_h.rearrange("(t p) -> p t", p=128))
            ub2c = pp.tile([128, 8], F32)
            dma.dma_start(out=ub2c, in_=ub2_h.rearrange("(t p) -> p t", p=128))

            g_bc = pp.tile([128, D], F32)
            b_bc = pp.tile([128, D], F32)
            lr_t = pp.tile([1, 1], F32)
            dma.dma_start(out=lr_t, in_=lr_h[:])
            lra = pp.tile([1, 1], F32)
            nc.scalar.activation(lra, lr_t, AF.Abs)
            lrn_col = pp.tile([128, 1], F32)

            with tc.tile_pool(name="bcps", bufs=2, space="PSUM") as bps, \
                 tc.tile_pool(name="bcrows", bufs=1) as brp:
                lng_row = brp.tile([1, D], F32)
                dma.dma_start(out=lng_row, in_=lng_h.rearrange("(o d) -> o d", o=1))
                lnb_row = brp.tile([1, D], F32)
                dma.dma_start(out=lnb_row, in_=lnb_h.rearrange("(o d) -> o d", o=1))
                for n2 in range(2):
                    ps = bps.tile([128, 512], F32, tag="bc")
                    nc.tensor.matmul(ps, ones1f, lng_row[:, n2 * 512:(n2 + 1) * 512],
                                     start=True, stop=True)
                    nc.vector.tensor_copy(g_bc[:, n2 * 512:(n2 + 1) * 512], ps)
                    ps2 = bps.tile([128, 512], F32, tag="bc")
                    nc.tensor.matmul(ps2, ones1f, lnb_row[:, n2 * 512:(n2 + 1) * 512],
                                     start=True, stop=True)
                    nc.vector.tensor_copy(b_bc[:, n2 * 512:(n2 + 1) * 512], ps2)
                pl = bps.tile([128, 1], F32, tag="bc")
                nc.tensor.matmul(pl, ones1f, lra, start=True, stop=True)
                nc.vector.tensor_scalar_mul(lrn_col, pl, -1.0)

            db1a = pp.tile([128, 32], F32)
            nc.vector.memset(db1a, 0.0)
            db2a = pp.tile([128, 8], F32)
            nc.vector.memset(db2a, 0.0)

            # ------- setup: xs/tg casts to bf16 token-major scratch -------
            with tc.tile_pool(name="setup", bufs=3) as sp:
                for t in range(TTILES):
                    xt = sp.tile([128, D], F32, tag="ldf32")
                    dma.dma_start(out=xt, in_=xs_h[t * 128:(t + 1) * 128, :])
                    xb = sp.tile([128, D], BF16, tag="ldbf")
                    nc.vector.tensor_copy(xb, xt)
                    dma.dma_start(out=xs_t_s[t * 128:(t + 1) * 128, :], in_=xb)
                for t in range(TTILES):
                    xt = sp.tile([128, D], F32, tag="ldf32")
                    dma.dma_start(out=xt, in_=tg_h[t * 128:(t + 1) * 128, :])
                    xb = sp.tile([128, D], BF16, tag="ldbf")
                    nc.vector.tensor_copy(xb, xt)
                    dma.dma_start(out=tg_t_s[t * 128:(t + 1) * 128, :], in_=xb)

            # =================== PHASE A: P1+P2 ===================
            with tc.tile_pool(name="p12res", bufs=1) as rp, \
                 tc.tile_pool(name="p12", bufs=2) as wp, \
                 tc.tile_pool(name="p12u", bufs=2) as up, \
                 tc.tile_pool(name="p12ld", bufs=2) as sp, \
                 tc.tile_pool(name="p12ps", bufs=4, space="PSUM") as psp:
                w1 = rp.tile([128, 8, D4], BF16)
                for k in range(8):
                    for q in range(4):
                        wt = sp.tile([128, D], F32, tag="ldw")
                        dma.dma_start(out=wt, in_=W1_h[k * 128:(k + 1) * 128, q * D:(q + 1) * D])
                        nc.vector.tensor_copy(w1[:, k, q * D:(q + 1) * D], wt)
                w2 = rp.tile([128, 16, D], BF16)
                for k in range(16):
                    wt = sp.tile([128, D], F32, tag="ldw")
                    dma.dma_start(out=wt, in_=W2_h[k * 128:(k + 1) * 128, :])
                    nc.vector.tensor_copy(w2[:, k, :], wt)
                    dma.dma_start(out=w2_s[k * 128:(k + 1) * 128, :], in_=w2[:, k, :])
                xs_f = rp.tile([128, 8, N], BF16)
                for k in range(8):
                    dma.dma_start_transpose(xs_f[:, k, :], xs_t_s[:, k * 128:(k + 1) * 128])

                for c in range(NCHUNK):
                    cs = slice(c * NT, (c + 1) * NT)
                    u_ch = up.tile([128, 16, NT], BF16, tag="u_ch")
                    for j in range(16):
                        psA = psp.tile([128, NT], F32, tag="ps")
                        for k in range(8):
                            nc.tensor.matmul(psA, w1[:, k, j * 128:(j + 1) * 128],
                                             xs_f[:, k, cs], start=(k == 0), stop=(k == 7))
                        psG = psp.tile([128, NT], F32, tag="ps")
                        for k in range(8):
                            nc.tensor.matmul(psG, w1[:, k, D2 + j * 128:D2 + (j + 1) * 128],
                                             xs_f[:, k, cs], start=(k == 0), stop=(k == 7))
                        s_sb = wp.tile([128, NT], BF16, tag="s_sb")
                        nc.scalar.activation(s_sb, psA, AF.Silu, bias=b1c[:, j:j + 1])
                        sgp_sb = wp.tile([128, NT], BF16, tag="sgp_sb")
                        nc.scalar.activation(sgp_sb, psA, AF.Derivative_silu, bias=b1c[:, j:j + 1])
                        gt_sb = wp.tile([128, NT], BF16, tag="gt_sb")
                        nc.scalar.activation(gt_sb, psG, AF.Identity, bias=b1c[:, 16 + j:17 + j])
                        nc.vector.tensor_tensor(out=u_ch[:, j, :], in0=s_sb, in1=gt_sb, op=ALU.mult)
                        dma.dma_start(out=s_f_s[j * 128:(j + 1) * 128, cs], in_=s_sb)
                        dma.dma_start(out=sgp_f_s[j * 128:(j + 1) * 128, cs], in_=sgp_sb)
                        dma.dma_start(out=gt_f_s[j * 128:(j + 1) * 128, cs], in_=gt_sb)
                        dma.dma_start(out=u_f_s[j * 128:(j + 1) * 128, cs], in_=u_ch[:, j, :])
                    for m in range(8):
                        psH = psp.tile([128, NT], F32, tag="ps")
                        for k in range(16):
                            nc.tensor.matmul(psH, w2[:, k, m * 128:(m + 1) * 128],
                                             u_ch[:, k, :], start=(k == 0), stop=(k == 15))
                        h_sb = wp.tile([128, NT], BF16, tag="h_sb")
                        nc.scalar.activation(h_sb, psH, AF.Identity, bias=b2c[:, m:m + 1])
                        dma.dma_start(out=h_f_s[m * 128:(m + 1) * 128, cs], in_=h_sb)

            # =================== PHASE A: P3a (pred fwd + loss -> dpr) ===================
            with tc.tile_pool(name="p3res", bufs=1) as rp, \
                 tc.tile_pool(name="p3", bufs=2) as wp, \
                 tc.tile_pool(name="p3c", bufs=1) as cwp, \
                 tc.tile_pool(name="p3loc", bufs=1) as lp, \
                 tc.tile_pool(name="p3ld", bufs=2) as sp, \
                 tc.tile_pool(name="p3ps", bufs=4, space="PSUM") as psp, \
                 tc.tile_pool(name="p3ps2", bufs=1, space="PSUM") as psp2:
                pw1 = rp.tile([128, 8, D2], BF16)
                for k in range(8):
                    for q in range(2):
                        wt = sp.tile([128, D], F32, tag="ldw")
                        dma.dma_start(out=wt, in_=pW1_h[k * 128:(k + 1) * 128, q * D:(q + 1) * D])
                        nc.vector.tensor_copy(pw1[:, k, q * D:(q + 1) * D], wt)
                    dma.dma_start(out=pw1_s[k * 128:(k + 1) * 128, :], in_=pw1[:, k, :])
                pw2 = rp.tile([128, 8, D], BF16)
                for k in range(8):
                    wt = sp.tile([128, D], F32, tag="ldw")
                    dma.dma_start(out=wt, in_=pW2_h[k * 128:(k + 1) * 128, :])
                    nc.vector.tensor_copy(pw2[:, k, :], wt)
                    dma.dma_start(out=pw2_s[k * 128:(k + 1) * 128, :], in_=pw2[:, k, :])

                for c in range(NCHUNK):
                    cs = slice(c * NT, (c + 1) * NT)
                    h_ch = lp.tile([128, 8, NT], BF16, tag="h_ch")
                    dma.dma_start(out=h_ch, in_=h_f_s.rearrange("(m p) n -> p m n", p=128)[:, :, cs])
                    tg_ch = lp.tile([128, 8, NT], BF16, tag="tg_ch")
                    for k in range(8):
                        dma.dma_start_transpose(tg_ch[:, k, :], tg_t_s[cs, k * 128:(k + 1) * 128])
                    tn2c = cwp.tile([16, NT], F32, tag="tn2c")
                    psT = psp2.tile([16, NT], F32, tag="psT")
                    for j in range(8):
                        sq = wp.tile([128, NT], BF16, tag="tgsq")
                        nc.scalar.activation(sq, tg_ch[:, j, :], AF.Square)
                        nc.tensor.matmul(psT, ind[:, j, :], sq, start=(j == 0), stop=(j == 7))
                    nc.vector.tensor_copy(tn2c, psT)

                    s2_ch = lp.tile([128, 8, NT], BF16, tag="s2_ch")
                    g2_ch = lp.tile([128, 8, NT], BF16, tag="g2_ch")
                    u2_ch = lp.tile([128, 8, NT], BF16, tag="u2_ch")
                    for j in range(8):
                        psA = psp.tile([128, NT], F32, tag="ps")
                        for k in range(8):
                            nc.tensor.matmul(psA, pw1[:, k, j * 128:(j + 1) * 128],
                                             h_ch[:, k, :], start=(k == 0), stop=(k == 7))
                        psG = psp.tile([128, NT], F32, tag="ps")
                        for k in range(8):
                            nc.tensor.matmul(psG, pw1[:, k, D + j * 128:D + (j + 1) * 128],
                                             h_ch[:, k, :], start=(k == 0), stop=(k == 7))
                        nc.scalar.activation(s2_ch[:, j, :], psA, AF.Silu, bias=pb1c[:, j:j + 1])
                        sgp_sb = wp.tile([128, NT], BF16, tag="sgp2")
                        nc.scalar.activation(sgp_sb, psA, AF.Derivative_silu, bias=pb1c[:, j:j + 1])
                        dma.dma_start(out=sgp2_f_s[j * 128:(j + 1) * 128, cs], in_=sgp_sb)
                        nc.scalar.activation(g2_ch[:, j, :], psG, AF.Identity, bias=pb1c[:, 8 + j:9 + j])
                        nc.vector.tensor_tensor(out=u2_ch[:, j, :], in0=s2_ch[:, j, :],
                                                in1=g2_ch[:, j, :], op=ALU.mult)
                        dma.dma_start(out=s2_f_s[j * 128:(j + 1) * 128, cs], in_=s2_ch[:, j, :])
                        dma.dma_start(out=g2_f_s[j * 128:(j + 1) * 128, cs], in_=g2_ch[:, j, :])
                    pr_ch = lp.tile([128, 8, NT], BF16, tag="pr_ch")
                    psN = psp2.tile([16, NT], F32, tag="psN")
                    psP = psp2.tile([16, NT], F32, tag="psP")
                    for m in range(8):
                        psPR = psp.tile([128, NT], F32, tag="ps")
                        for k in range(8):
                            nc.tensor.matmul(psPR, pw2[:, k, m * 128:(m + 1) * 128],
                                             u2_ch[:, k, :], start=(k == 0), stop=(k == 7))
                        nc.scalar.activation(pr_ch[:, m, :], psPR, AF.Identity, bias=pb2c[:, m:m + 1])
                        prtg = wp.tile([128, NT], BF16, tag="prtg")
                        nc.vector.tensor_tensor(out=prtg, in0=pr_ch[:, m, :], in1=tg_ch[:, m, :], op=ALU.mult)
                        pr2 = wp.tile([128, NT], BF16, tag="pr2")
                        nc.scalar.activation(pr2, pr_ch[:, m, :], AF.Square)
                        nc.tensor.matmul(psN, ind[:, m, :], prtg, start=(m == 0), stop=(m == 7))
                        nc.tensor.matmul(psP, ind[:, m, :], pr2, start=(m == 0), stop=(m == 7))
                    tden = cwp.tile([16, NT], F32, tag="tden")
                    nc.vector.tensor_tensor(out=tden, in0=psP, in1=tn2c, op=ALU.mult)
                    nc.vector.tensor_scalar_max(tden, tden, 1e-16)
                    den = cwp.tile([16, NT], F32, tag="den")
                    nc.scalar.activation(den, tden, AF.Sqrt)
                    rden = cwp.tile([16, NT], F32, tag="rden")
                    nc.vector.reciprocal(rden, den)
                    qpn = cwp.tile([16, NT], F32, tag="qpn")
                    nc.vector.reciprocal(qpn, psP)
                    c1b = cwp.tile([16, NT], BF16, tag="c1b")
                    nc.vector.tensor_scalar_mul(c1b, rden, -1.0 / N)
                    m1 = cwp.tile([16, NT], F32, tag="m1")
                    nc.vector.tensor_tensor(out=m1, in0=psN, in1=rden, op=ALU.mult)
                    m2 = cwp.tile([16, NT], F32, tag="m2")
                    nc.vector.tensor_tensor(out=m2, in0=m1, in1=qpn, op=ALU.mult)
                    c2b = cwp.tile([16, NT], BF16, tag="c2b")
                    nc.vector.tensor_scalar_mul(c2b, m2, 1.0 / N)
                    for j in range(8):
                        psC1 = psp.tile([128, NT], F32, tag="ps")
                        nc.tensor.matmul(psC1, indt[:, j, :], c1b, start=True, stop=True)
                        psC2 = psp.tile([128, NT], F32, tag="ps")
                        nc.tensor.matmul(psC2, indt[:, j, :], c2b, start=True, stop=True)
                        t1 = wp.tile([128, NT], BF16, tag="t1")
                        nc.vector.tensor_tensor(out=t1, in0=tg_ch[:, j, :], in1=psC1, op=ALU.mult)
                        t2 = wp.tile([128, NT], BF16, tag="t2")
                        nc.vector.tensor_tensor(out=t2, in0=pr_ch[:, j, :], in1=psC2, op=ALU.mult)
                        dpr_sb = wp.tile([128, NT], BF16, tag="dpr_sb")
                        nc.vector.tensor_tensor(out=dpr_sb, in0=t1, in1=t2, op=ALU.add)
                        dma.dma_start(out=dpr_f_s[j * 128:(j + 1) * 128, cs], in_=dpr_sb)

            # =================== PHASE A: P3b (pred bwd -> dh, db2) ===================
            with tc.tile_pool(name="p3bres", bufs=1) as rp, \
                 tc.tile_pool(name="p3b", bufs=2) as wp, \
                 tc.tile_pool(name="p3bloc", bufs=1) as lp, \
                 tc.tile_pool(name="p3bps", bufs=4, space="PSUM") as psp:
                pw2T = rp.tile([128, 8, D], BF16)
                for k in range(8):
                    dma.dma_start_transpose(pw2T[:, k, :], pw2_s[:, k * 128:(k + 1) * 128])
                pw1T = rp.tile([128, 16, D], BF16)
                for k in range(16):
                    dma.dma_start_transpose(pw1T[:, k, :], pw1_s[:, k * 128:(k + 1) * 128])
                if DEBUG:
                    dbgout("pw2T", [128, 8 * D])
                    dma.dma_start(out=dbg["pw2T"][:], in_=pw2T.rearrange("p k d -> p (k d)"))
                    dbgout("pw1T", [128, 16 * D])
                    dma.dma_start(out=dbg["pw1T"][:], in_=pw1T.rearrange("p k d -> p (k d)"))
                dH2_dbg = nc.dram_tensor("dH2_dbg", [D2, N], BF16)
                for c in range(NCHUNK):
                    cs = slice(c * NT, (c + 1) * NT)
                    dpr_ch = lp.tile([128, 8, NT], BF16, tag="dpr_ch")
                    dma.dma_start(out=dpr_ch, in_=dpr_f_s.rearrange("(m p) n -> p m n", p=128)[:, :, cs])
                    s2_ch = lp.tile([128, 8, NT], BF16, tag="s2_ch")
                    dma.dma_start(out=s2_ch, in_=s2_f_s.rearrange("(m p) n -> p m n", p=128)[:, :, cs])
                    g2_ch = lp.tile([128, 8, NT], BF16, tag="g2_ch")
                    dma.dma_start(out=g2_ch, in_=g2_f_s.rearrange("(m p) n -> p m n", p=128)[:, :, cs])
                    sgp2_ch = lp.tile([128, 8, NT], BF16, tag="sgp2_ch")
                    dma.dma_start(out=sgp2_ch, in_=sgp2_f_s.rearrange("(m p) n -> p m n", p=128)[:, :, cs])
                    dH2_ch = lp.tile([128, 16, NT], BF16, tag="dH2_ch")
                    for m in range(8):
                        psDU2 = psp.tile([128, NT], F32, tag="ps")
                        for k in range(8):
                            nc.tensor.matmul(psDU2, pw2T[:, k, m * 128:(m + 1) * 128],
                                             dpr_ch[:, k, :], start=(k == 0), stop=(k == 7))
                        ds2 = wp.tile([128, NT], BF16, tag="ds2")
                        nc.vector.tensor_tensor(out=ds2, in0=psDU2, in1=g2_ch[:, m, :], op=ALU.mult)
                        nc.vector.tensor_tensor(out=dH2_ch[:, 8 + m, :], in0=psDU2,
                                                in1=s2_ch[:, m, :], op=ALU.mult)
                        nc.vector.tensor_tensor(out=dH2_ch[:, m, :], in0=ds2,
                                                in1=sgp2_ch[:, m, :], op=ALU.mult)
                    if DEBUG:
                        for mm_ in range(16):
                            dma.dma_start(out=dH2_dbg[mm_ * 128:(mm_ + 1) * 128, cs], in_=dH2_ch[:, mm_, :])
                    for m in range(8):
                        psDH = psp.tile([128, NT], F32, tag="ps")
                        for k in range(16):
                            nc.tensor.matmul(psDH, pw1T[:, k, m * 128:(m + 1) * 128],
                                             dH2_ch[:, k, :], start=(k == 0), stop=(k == 15))
                        dh_sb = wp.tile([128, NT], BF16, tag="dh_sb")
                        dbp = wp.tile([128, 1], F32, tag="dbp")
                        nc.vector.tensor_scalar(dh_sb, psDH, 1.0, 0.0, ALU.mult, ALU.add, accum_out=dbp)
                        nc.vector.tensor_tensor(out=db2a[:, m:m + 1], in0=db2a[:, m:m + 1],
                                                in1=dbp, op=ALU.add)
                        dma.dma_start(out=dh_f_s[m * 128:(m + 1) * 128, cs], in_=dh_sb)

            # =================== PHASE A: P4 ===================
            with tc.tile_pool(name="p4res", bufs=1) as rp, \
                 tc.tile_pool(name="p4", bufs=2) as wp, \
                 tc.tile_pool(name="p4loc", bufs=1) as lp, \
                 tc.tile_pool(name="p4ps", bufs=4, space="PSUM") as psp:
                w2T = rp.tile([128, 8, D2], BF16)
                for k in range(8):
                    dma.dma_start_transpose(w2T[:, k, :], w2_s[:, k * 128:(k + 1) * 128])
                for c in range(NCHUNK):
                    cs = slice(c * NT, (c + 1) * NT)
                    dh_ch = lp.tile([128, 8, NT], BF16, tag="dh_ch")
                    dma.dma_start(out=dh_ch, in_=dh_f_s.rearrange("(m p) n -> p m n", p=128)[:, :, cs])
                    s_ch = lp.tile([128, 16, NT], BF16, tag="s_ch")
                    dma.dma_start(out=s_ch, in_=s_f_s.rearrange("(m p) n -> p m n", p=128)[:, :, cs])
                    gt_ch = lp.tile([128, 16, NT], BF16, tag="gt_ch")
                    dma.dma_start(out=gt_ch, in_=gt_f_s.rearrange("(m p) n -> p m n", p=128)[:, :, cs])
                    sgp_ch = lp.tile([128, 16, NT], BF16, tag="sgp_ch")
                    dma.dma_start(out=sgp_ch, in_=sgp_f_s.rearrange("(m p) n -> p m n", p=128)[:, :, cs])
                    for m in range(16):
                        psDU = psp.tile([128, NT], F32, tag="ps")
                        for k in range(8):
                            nc.tensor.matmul(psDU, w2T[:, k, m * 128:(m + 1) * 128],
                                             dh_ch[:, k, :], start=(k == 0), stop=(k == 7))
                        ds = wp.tile([128, NT], BF16, tag="ds")
                        nc.vector.tensor_tensor(out=ds, in0=psDU, in1=gt_ch[:, m, :], op=ALU.mult)
                        da = wp.tile([128, NT], BF16, tag="da")
                        dbp = wp.tile([128, 1], F32, tag="dbp4")
                        nc.vector.scalar_tensor_tensor(da, ds, 1.0, sgp_ch[:, m, :],
                                                       ALU.mult, ALU.mult, accum_out=dbp)
                        nc.vector.tensor_tensor(out=db1a[:, m:m + 1], in0=db1a[:, m:m + 1],
                                                in1=dbp, op=ALU.add)
                        dgt = wp.tile([128, NT], BF16, tag="dgt")
                        dbp2 = wp.tile([128, 1], F32, tag="dbp4b")
                        nc.vector.scalar_tensor_tensor(dgt, psDU, 1.0, s_ch[:, m, :],
                                                       ALU.mult, ALU.mult, accum_out=dbp2)
                        nc.vector.tensor_tensor(out=db1a[:, 16 + m:17 + m], in0=db1a[:, 16 + m:17 + m],
                                                in1=dbp2, op=ALU.add)
                        dma.dma_start(out=dH1_f_s[m * 128:(m + 1) * 128, cs], in_=da)
                        dma.dma_start(out=dH1_f_s[D2 + m * 128:D2 + (m + 1) * 128, cs], in_=dgt)

            # =================== PHASE B ===================
            with tc.tile_pool(name="b1res", bufs=1) as rp, \
                 tc.tile_pool(name="b1w", bufs=3) as wp, \
                 tc.tile_pool(name="b1ps", bufs=4, space="PSUM") as psp:
                xs_tt = rp.tile([128, 16, D], BF16)
                dma.dma_start(out=xs_tt, in_=xs_t_s.rearrange("(t p) d -> p t d", p=128))
                dH1_t = rp.tile([128, 16, D4], BF16)
                for t in range(16):
                    dma.dma_start_transpose(dH1_t[:, t, :], dH1_f_s[:, t * 128:(t + 1) * 128])
                for m in range(8):
                    for n in range(8):
                        ps = psp.tile([128, 512], F32, tag="ps")
                        for k in range(16):
                            nc.tensor.matmul(ps, xs_tt[:, k, m * 128:(m + 1) * 128],
                                             dH1_t[:, k, n * 512:(n + 1) * 512],
                                             start=(k == 0), stop=(k == 15))
                        uw = wp.tile([128, 512], F32, tag="uw1")
                        dma.dma_start(out=uw, in_=uW1_h[m * 128:(m + 1) * 128, n * 512:(n + 1) * 512])
                        w1p_t = wp.tile([128, 512], BF16, tag="w1p")
                        nc.vector.scalar_tensor_tensor(w1p_t, ps, lrn_col, uw, ALU.mult, ALU.add)
                        dma.dma_start(out=w1p_s[m * 128:(m + 1) * 128, n * 512:(n + 1) * 512], in_=w1p_t)
            with tc.tile_pool(name="b2res", bufs=1) as rp, \
                 tc.tile_pool(name="b2w", bufs=3) as wp, \
                 tc.tile_pool(name="b2ps", bufs=4, space="PSUM") as psp:
                u_t = rp.tile([128, 16, D2], BF16)
                for t in range(16):
                    dma.dma_start_transpose(u_t[:, t, :], u_f_s[:, t * 128:(t + 1) * 128])
                dh_t = rp.tile([128, 16, D], BF16)
                for t in range(16):
                    dma.dma_start_transpose(dh_t[:, t, :], dh_f_s[:, t * 128:(t + 1) * 128])
                for m in range(16):
                    for n in range(2):
                        ps = psp.tile([128, 512], F32, tag="ps")
                        for k in range(16):
                            nc.tensor.matmul(ps, u_t[:, k, m * 128:(m + 1) * 128],
                                             dh_t[:, k, n * 512:(n + 1) * 512],
                                             start=(k == 0), stop=(k == 15))
                        uw = wp.tile([128, 512], F32, tag="uw2")
                        dma.dma_start(out=uw, in_=uW2_h[m * 128:(m + 1) * 128, n * 512:(n + 1) * 512])
                        w2p_t = wp.tile([128, 512], BF16, tag="w2p")
                        nc.vector.scalar_tensor_tensor(w2p_t, ps, lrn_col, uw, ALU.mult, ALU.add)
                        dma.dma_start(out=w2p_s[m * 128:(m + 1) * 128, n * 512:(n + 1) * 512], in_=w2p_t)
                b1p_t = wp.tile([128, 32], BF16, tag="b1p")
                nc.vector.scalar_tensor_tensor(b1p_t, db1a, lrn_col, ub1c, ALU.mult, ALU.add)
                dma.dma_start(out=b1p_s.rearrange("(t p) -> p t", p=128), in_=b1p_t)
                b2p_t = wp.tile([128, 8], BF16, tag="b2p")
                nc.vector.scalar_tensor_tensor(b2p_t, db2a, lrn_col, ub2c, ALU.mult, ALU.add)
                dma.dma_start(out=b2p_s.rearrange("(t p) -> p t", p=128), in_=b2p_t)

            # =================== PHASE C ===================
            with tc.tile_pool(name="cres", bufs=1) as rp, \
                 tc.tile_pool(name="cw", bufs=2) as wp, \
                 tc.tile_pool(name="cps", bufs=3, space="PSUM") as psp, \
                 tc.tile_pool(name="cpst", bufs=2, space="PSUM") as pst_p, \
                 tc.tile_pool(name="cpsy", bufs=1, space="PSUM") as pspy:
                w1p = rp.tile([128, 8, D4], BF16)
                dma.dma_start(out=w1p, in_=w1p_s.rearrange("(k p) f -> p k f", p=128))
                w2p = rp.tile([128, 16, D], BF16)
                dma.dma_start(out=w2p, in_=w2p_s.rearrange("(k p) f -> p k f", p=128))
                xs_f2 = rp.tile([128, 8, N], BF16)
                for k in range(8):
                    dma.dma_start_transpose(xs_f2[:, k, :], xs_t_s[:, k * 128:(k + 1) * 128])
                b1p_row = rp.tile([1, D4], BF16)
                dma.dma_start(out=b1p_row, in_=b1p_s.rearrange("(o f) -> o f", o=1))
                b2p_row = rp.tile([1, D], BF16)
                dma.dma_start(out=b2p_row, in_=b2p_s.rearrange("(o f) -> o f", o=1))
                for t in range(TTILES):
                    ts_ = slice(t * 128, (t + 1) * 128)
                    up_t = wp.tile([128, D2], BF16, tag="up_t")
                    for n in range(4):
                        psA = psp.tile([128, 512], F32, tag="cps")
                        nc.tensor.matmul(psA, ones1b, b1p_row[:, n * 512:(n + 1) * 512],
                                         start=True, stop=False)
                        for k in range(8):
                            nc.tensor.matmul(psA, xs_f2[:, k, ts_],
                                             w1p[:, k, n * 512:(n + 1) * 512],
                                             start=False, stop=(k == 7))
                        psG = psp.tile([128, 512], F32, tag="cps")
                        nc.tensor.matmul(psG, ones1b, b1p_row[:, D2 + n * 512:D2 + (n + 1) * 512],
                                         start=True, stop=False)
                        for k in range(8):
                            nc.tensor.matmul(psG, xs_f2[:, k, ts_],
                                             w1p[:, k, D2 + n * 512:D2 + (n + 1) * 512],
                                             start=False, stop=(k == 7))
                        sp_sb = wp.tile([128, 512], BF16, tag="sp_sb")
                        nc.scalar.activation(sp_sb, psA, AF.Silu)
                        nc.vector.tensor_tensor(out=up_t[:, n * 512:(n + 1) * 512],
                                                in0=sp_sb, in1=psG, op=ALU.mult)
                    upf = wp.tile([128, 16, 128], BF16, tag="upf")
                    for q in range(16):
                        pst = pst_p.tile([128, 128], BF16, tag="cpst")
                        nc.tensor.transpose(pst, up_t[:, q * 128:(q + 1) * 128], ident)
                        nc.vector.tensor_copy(upf[:, q, :], pst)
                    psY = pspy.tile([128, D], F32, tag="psY")
                    for n2 in range(2):
                        ys = slice(n2 * 512, (n2 + 1) * 512)
                        nc.tensor.matmul(psY[:, ys], ones1b, b2p_row[:, ys], start=True, stop=False)
                        for k in range(16):
                            nc.tensor.matmul(psY[:, ys], upf[:, k, :],
                                             w2p[:, k, ys], start=False, stop=(k == 15))
                    mu = wp.tile([128, 1], F32, tag="mu")
                    nc.vector.tensor_reduce(mu, psY, AX.X, ALU.add)
                    nc.vector.tensor_scalar_mul(mu, mu, 1.0 / D)
                    sq = wp.tile([128, D], F32, tag="sq")
                    nc.scalar.activation(sq, psY, AF.Square)
                    msq = wp.tile([128, 1], F32, tag="msq")
                    nc.vector.tensor_reduce(msq, sq, AX.X, ALU.add)
                    mu2 = wp.tile([128, 1], F32, tag="mu2")
                    nc.vector.tensor_tensor(out=mu2, in0=mu, in1=mu, op=ALU.mult)
                    var = wp.tile([128, 1], F32, tag="var")
                    nc.vector.tensor_scalar(var, msq, 1.0 / D, None, ALU.mult)
                    nc.vector.tensor_tensor(out=var, in0=var, in1=mu2, op=ALU.subtract)
                    nc.vector.tensor_scalar_add(var, var, 1e-5)
                    sd = wp.tile([128, 1], F32, tag="sd")
                    nc.scalar.activation(sd, var, AF.Sqrt)
                    rstd = wp.tile([128, 1], F32, tag="rstd")
                    nc.vector.reciprocal(rstd, sd)
                    yn = wp.tile([128, D], F32, tag="yn")
                    nc.vector.tensor_scalar(yn, psY, mu, rstd, ALU.subtract, ALU.mult)
                    yo = wp.tile([128, D], F32, tag="yo")
                    nc.vector.tensor_tensor(out=yo, in0=yn, in1=g_bc, op=ALU.mult)
                    nc.vector.tensor_tensor(out=yo, in0=yo, in1=b_bc, op=ALU.add)
                    dma.dma_start(out=Y[ts_, :], in_=yo)

            if DEBUG:
                for nm, t in [("xs_t", xs_t_s), ("u_f", u_f_s), ("h_f", h_f_s),
                              ("s_f", s_f_s), ("gt_f", gt_f_s), ("sgp_f", sgp_f_s),
                              ("dpr_f", dpr_f_s), ("dh_f", dh_f_s), ("dH1_f", dH1_f_s),
                              ("w1p", w1p_s), ("w2p", w2p_s),
                              ("s2_f", s2_f_s), ("g2_f", g2_f_s)]:
                    dbgout(nm, t.shape)
                    dma.dma_start(out=dbg[nm][:], in_=t[:])
                dbgout("b1p", [D4]); dma.dma_start(out=dbg["b1p"][:], in_=b1p_s[:])
                dbgout("sgp2_f", [D, N]); dma.dma_start(out=dbg["sgp2_f"][:], in_=sgp2_f_s[:])
                dbgout("dH2", [D2, N]); dma.dma_start(out=dbg["dH2"][:], in_=dH2_dbg[:])
                dbgout("b2p", [D]); dma.dma_start(out=dbg["b2p"][:], in_=b2p_s[:])

    tile_fix.split_sync(nc)
    return nc


_NC_CACHE = {}
LAST_RESULTS = None


def kernel(**inputs):
    x = np.asarray(inputs["x"], dtype=np.float32)
    tg = np.asarray(inputs["targets"], dtype=np.float32).reshape(B, N, G * GD)
    ident, ones1b, ones1f, ind, indt = _consts()
    common = {
        "W1": np.asarray(inputs["in_W1"], np.float32),
        "b1": np.asarray(inputs["in_b1"], np.float32),
        "W2": np.asarray(inputs["in_W2"], np.float32),
        "b2": np.asarray(inputs["in_b2"], np.float32),
        "pW1": np.asarray(inputs["pr_W1"], np.float32),
        "pb1": np.asarray(inputs["pr_b1"], np.float32),
        "pW2": np.asarray(inputs["pr_W2"], np.float32),
        "pb2": np.asarray(inputs["pr_b2"], np.float32),
        "uW1": np.asarray(inputs["u_W1"], np.float32),
        "ub1": np.asarray(inputs["u_b1"], np.float32),
        "uW2": np.asarray(inputs["u_W2"], np.float32),
        "ub2": np.asarray(inputs["u_b2"], np.float32),
        "lr": np.asarray(inputs["lr"], np.float32).reshape(1, 1),
        "lng": np.asarray(inputs["ln_g"], np.float32),
        "lnb": np.asarray(inputs["ln_b"], np.float32),
        "ident": ident, "ones1b": ones1b, "ones1f": ones1f,
        "ind": ind.reshape(8 * 128, 16), "indt": indt.reshape(8 * 16, 128),
    }
    n_cores = int(os.environ.get("KERNEL_CORES", "8"))
    in_maps = []
    for i in range(n_cores):
        m = dict(common)
        m["xs"] = np.ascontiguousarray(x[i % B])
        m["tg"] = np.ascontiguousarray(tg[i % B])
        in_maps.append(m)
    if "nc" not in _NC_CACHE:
        _NC_CACHE["nc"] = build_nc()
    nc = _NC_CACHE["nc"]
    res = run_bass_kernel_spmd(nc, in_maps, list(range(n_cores)))
    global LAST_RESULTS
    LAST_RESULTS = res.results
    ys = [res.results[i]["y"] for i in range(n_cores)]
    if n_cores >= B:
        y = np.stack(ys[:B])
    else:
        y = np.stack([ys[i % n_cores] for i in range(B)])
    return y.astype(np.float32)
